# revision 9
# baseline (speedup 1.0000x reference)
"""Trainium2 Bass kernel for nn_Affinity1d (gnn_message_passing).

Math (see original module): with w_e, w_t, w_p = split(Wcat),
    out[b, 0, i, j] = sum_e w_e[e] * edges[b, e, i, j]
                    + (w_t @ Wt @ x[b])[i]       # s_t, varies over rows
                    + (w_p @ Wp @ x[b])[j]       # s_p, varies over cols
`adj` only contributes its spatial size -> never shipped to the device.

Sharding: data-parallel over batch B=8 across the 8 NeuronCores (one
batch per core); the tiny folded weights are replicated.

Per-core device kernel:
  - s_t, s_p computed on PE from x (fp32, exact): v.T @ x as K=128x2
    accumulating matmuls, then a K=1 ones-matmul broadcasts s_p across
    partitions and a DRAM-roundtrip DMA transposes s_t into per-partition
    columns.
  - The dominant term streams edges (cast to fp16 on host: rel err ~1e-4,
    halves HBM traffic) in 1 MiB DMAs and reduces over the E=16 channels
    on the tensor engine: 16 PSUM-accumulating matmuls per output tile
    with scaled-identity stationary weights (out += w_e * I @ tile_e).
  - One DVE scalar_tensor_tensor pass per tile fuses
    out = psum + s_t[per-partition] + s_p[broadcast row].
"""

import sys

if "/opt/trn_rl_repo" not in sys.path:
    sys.path.insert(0, "/opt/trn_rl_repo")

import numpy as np

from concourse import bacc, bass, mybir, tile
from concourse.bass_utils import run_bass_kernel_spmd

B, H, NIN, C, E = 8, 1024, 256, 128, 16
N_CORES = 8
P = 128          # partitions / rows per output chunk
NCHUNK = H // P  # 8 row-chunks per core
EG = 4           # edge channels per DMA (1 MiB fp16 transfers)
FD = 512         # matmul free dim (one PSUM bank of fp32)

F32 = mybir.dt.float32
F16 = mybir.dt.float16

_CACHED = None


def _build_program():
    nc = bacc.Bacc("TRN2", debug=False, num_devices=N_CORES)

    # host-relayouted: [chunk, group, row, e_local, col] so each (chunk, group)
    # DMA reads 8 KiB fully-contiguous per partition row
    edges_d = nc.dram_tensor(
        "edges", [NCHUNK, E // EG, P, EG, H], F16, kind="ExternalInput"
    )
    x_d = nc.dram_tensor("x", [NIN, H], F32, kind="ExternalInput")
    vt_d = nc.dram_tensor("vt", [NIN, 1], F32, kind="ExternalInput")
    vp_d = nc.dram_tensor("vp", [NIN, 1], F32, kind="ExternalInput")
    wid_d = nc.dram_tensor("wid", [P, E, P], F16, kind="ExternalInput")
    out_d = nc.dram_tensor("out", [H, H], F32, kind="ExternalOutput")

    st_scratch = nc.dram_tensor("st_scratch", [1, H], F32)

    add = mybir.AluOpType.add

    with tile.TileContext(nc) as tc:
        with (
            tc.tile_pool(name="const", bufs=1) as const,
            tc.tile_pool(name="setup_psum", bufs=1, space="PSUM") as spsum,
            tc.tile_pool(name="edges", bufs=16) as epool,
            tc.tile_pool(name="outs", bufs=3) as opool,
            tc.tile_pool(name="mpsum", bufs=2, space="PSUM") as mpsum,
        ):
            # ---- constants / setup ----
            wid = const.tile([P, E, P], F16, tag="wid")
            nc.gpsimd.dma_start(wid[:], wid_d[:])

            x0 = const.tile([P, H], F32, tag="x0")
            x1 = const.tile([P, H], F32, tag="x1")
            nc.gpsimd.dma_start(x0[:], x_d[0:P, :])
            nc.gpsimd.dma_start(x1[:], x_d[P : 2 * P, :])

            vt0 = const.tile([P, 1], F32, tag="vt0")
            vt1 = const.tile([P, 1], F32, tag="vt1")
            vp0 = const.tile([P, 1], F32, tag="vp0")
            vp1 = const.tile([P, 1], F32, tag="vp1")
            nc.gpsimd.dma_start(vt0[:], vt_d[0:P, :])
            nc.gpsimd.dma_start(vt1[:], vt_d[P : 2 * P, :])
            nc.gpsimd.dma_start(vp0[:], vp_d[0:P, :])
            nc.gpsimd.dma_start(vp1[:], vp_d[P : 2 * P, :])

            ones = const.tile([1, P], F32, tag="ones")
            nc.gpsimd.memset(ones[:], 1.0)

            # s_t / s_p rows: (1, H) = v.T @ x, K=256 split into 2 matmuls
            st_row = const.tile([1, H], F32, tag="st_row")
            sp_row = const.tile([1, H], F32, tag="sp_row")
            for row, v0, v1 in ((st_row, vt0, vt1), (sp_row, vp0, vp1)):
                for jh in range(2):
                    ps = spsum.tile([1, FD], F32, tag="sps")
                    sl = slice(jh * FD, (jh + 1) * FD)
                    nc.tensor.matmul(ps[:], v0[:], x0[:, sl], start=True, stop=False)
                    nc.tensor.matmul(ps[:], v1[:], x1[:, sl], start=False, stop=True)
                    nc.vector.tensor_copy(row[:, sl], ps[:])

            # s_t as per-partition columns: (P, NCHUNK), st_cols[p, c] = s_t[c*P+p]
            st_cols = const.tile([P, NCHUNK], F32, tag="st_cols")
            nc.gpsimd.dma_start(st_scratch[:], st_row[:])
            nc.gpsimd.dma_start(
                st_cols[:],
                st_scratch[:].rearrange("o (c p) -> (o p) c", p=P),
            )

            # s_p broadcast across partitions: (P, H)
            sp_rep = const.tile([P, H], F32, tag="sp_rep")
            for jh in range(2):
                pb = spsum.tile([P, FD], F32, tag="spb")
                sl = slice(jh * FD, (jh + 1) * FD)
                nc.tensor.matmul(pb[:], ones[:], sp_row[:, sl], start=True, stop=True)
                nc.vector.tensor_copy(sp_rep[:, sl], pb[:])

            # ---- main loop: stream edges, PSUM-accumulate over E ----
            for c in range(NCHUNK):
                rows = slice(c * P, (c + 1) * P)
                etiles = []
                for g in range(E // EG):
                    t = epool.tile([P, EG, H], F16, tag="edge")
                    dma_eng = nc.sync if (c * (E // EG) + g) % 2 == 0 else nc.scalar
                    dma_eng.dma_start(t[:], edges_d[c, g])
                    etiles.append(t)

                pss = [
                    mpsum.tile([P, FD], F32, name=f"ps{jh}", tag=f"ps{jh}")
                    for jh in range(2)
                ]
                # e-outer / jh-inner: consecutive matmul pairs share the
                # stationary weights, so the next LDWEIGHTS hides under the
                # paired matmul. Groups for the two PSUM banks interleave.
                for e in range(E):
                    for jh in range(2):
                        sl = slice(jh * FD, (jh + 1) * FD)
                        nc.tensor.matmul(
                            pss[jh][:],
                            wid[:, e, :],
                            etiles[e // EG][:, e % EG, sl],
                            start=(e == 0),
                            stop=(e == E - 1),
                            skip_group_check=True,
                        )

                # split the combine+store per half so the final store drains
                # as soon as its bank's accumulation finishes
                for jh in range(2):
                    sl = slice(jh * FD, (jh + 1) * FD)
                    oth = opool.tile([P, FD], F32, name=f"ot{jh}", tag=f"ot{jh}")
                    nc.vector.scalar_tensor_tensor(
                        out=oth[:],
                        in0=pss[jh][:],
                        scalar=st_cols[:, c : c + 1],
                        in1=sp_rep[:, sl],
                        op0=add,
                        op1=add,
                    )
                    nc.gpsimd.dma_start(out_d[rows, sl], oth[:])

    nc.compile()
    return nc


def _get_program():
    global _CACHED
    if _CACHED is None:
        _CACHED = _build_program()
    return _CACHED


def kernel(adj, edges, x, Wt, Wp, Wcat, _trace=False):
    del adj  # only its spatial size matters; unused numerically

    edges = np.asarray(edges, dtype=np.float32)
    x = np.asarray(x, dtype=np.float32)
    Wt = np.asarray(Wt, dtype=np.float32)
    Wp = np.asarray(Wp, dtype=np.float32)
    Wcat = np.asarray(Wcat, dtype=np.float32)

    # Fold the 1x1-conv weights: the theta/phi paths collapse to vectors.
    w_e = Wcat[:E]
    v_t = (Wcat[E : E + C] @ Wt).astype(np.float32).reshape(NIN, 1)
    v_p = (Wcat[E + C :] @ Wp).astype(np.float32).reshape(NIN, 1)

    eye = np.eye(P, dtype=np.float32)
    wid = (eye[:, None, :] * w_e[None, :, None]).astype(np.float16)  # (P, E, P)

    # cast to fp16 and relayout to [chunk, group, row, e_local, col] so every
    # device DMA reads fully-contiguous 8 KiB per partition row
    edges16 = edges.astype(np.float16)
    edges16 = edges16.reshape(B, E // EG, EG, NCHUNK, P, H).transpose(0, 3, 1, 4, 2, 5)

    in_maps = []
    for b in range(B):
        in_maps.append(
            {
                "edges": np.ascontiguousarray(edges16[b]),
                "x": np.ascontiguousarray(x[b]),
                "vt": v_t,
                "vp": v_p,
                "wid": wid,
            }
        )

    nc = _get_program()
    res = run_bass_kernel_spmd(nc, in_maps, list(range(N_CORES)), trace=_trace)
    global LAST_RESULT
    LAST_RESULT = res

    out = np.stack([res.results[b]["out"] for b in range(B)])
    return out[:, None, :, :].astype(np.float32)


LAST_RESULT = None


# revision 11
# speedup vs baseline: 1.1160x; 1.1160x over previous
"""Trainium2 Bass kernel for nn_Affinity1d (gnn_message_passing).

Math (see original module): with w_e, w_t, w_p = split(Wcat),
    out[b, 0, i, j] = sum_e w_e[e] * edges[b, e, i, j]
                    + (w_t @ Wt @ x[b])[i]       # s_t, varies over rows
                    + (w_p @ Wp @ x[b])[j]       # s_p, varies over cols
`adj` only contributes its spatial size -> never shipped to the device.

Sharding: data-parallel over batch B=8 across the 8 NeuronCores (one
batch per core); the tiny folded weights are replicated.

Per-core device kernel:
  - s_t, s_p computed on PE from x (fp32, exact): v.T @ x as K=128x2
    accumulating matmuls, then a K=1 ones-matmul broadcasts s_p across
    partitions and a DRAM-roundtrip DMA transposes s_t into per-partition
    columns.
  - The dominant term streams edges (cast to fp16 on host: rel err ~1e-4,
    halves HBM traffic) in 1 MiB DMAs and reduces over the E=16 channels
    on the tensor engine: 16 PSUM-accumulating matmuls per output tile
    with scaled-identity stationary weights (out += w_e * I @ tile_e).
  - One DVE scalar_tensor_tensor pass per tile fuses
    out = psum + s_t[per-partition] + s_p[broadcast row].
"""

import sys

if "/opt/trn_rl_repo" not in sys.path:
    sys.path.insert(0, "/opt/trn_rl_repo")

import numpy as np

from concourse import bacc, bass, mybir, tile
from concourse.bass_utils import run_bass_kernel_spmd

B, H, NIN, C, E = 8, 1024, 256, 128, 16
N_CORES = 8
P = 128          # partitions / rows per output chunk
NCHUNK = H // P  # 8 row-chunks per core
EG = 4           # edge channels per DMA (1 MiB fp16 transfers)
FD = 512         # matmul free dim (one PSUM bank of fp32)

F32 = mybir.dt.float32
F16 = mybir.dt.float16

_CACHED = None


def _build_program():
    nc = bacc.Bacc("TRN2", debug=False, num_devices=N_CORES)

    # host-relayouted: [chunk, group, row, e_local, col] so each (chunk, group)
    # DMA reads 8 KiB fully-contiguous per partition row
    edges_d = nc.dram_tensor(
        "edges", [NCHUNK, E // EG, P, EG, H], F16, kind="ExternalInput"
    )
    x_d = nc.dram_tensor("x", [NIN, H], F32, kind="ExternalInput")
    vt_d = nc.dram_tensor("vt", [NIN, 1], F32, kind="ExternalInput")
    vp_d = nc.dram_tensor("vp", [NIN, 1], F32, kind="ExternalInput")
    wid_d = nc.dram_tensor("wid", [P, E, P], F16, kind="ExternalInput")
    out_d = nc.dram_tensor("out", [H, H], F16, kind="ExternalOutput")

    st_scratch = nc.dram_tensor("st_scratch", [1, H], F32)

    add = mybir.AluOpType.add

    with tile.TileContext(nc) as tc:
        with (
            tc.tile_pool(name="const", bufs=1) as const,
            tc.tile_pool(name="setup_psum", bufs=1, space="PSUM") as spsum,
            tc.tile_pool(name="edges", bufs=16) as epool,
            tc.tile_pool(name="outs", bufs=3) as opool,
            tc.tile_pool(name="mpsum", bufs=2, space="PSUM") as mpsum,
        ):
            # ---- constants / setup ----
            wid = const.tile([P, E, P], F16, tag="wid")
            nc.gpsimd.dma_start(wid[:], wid_d[:])

            x0 = const.tile([P, H], F32, tag="x0")
            x1 = const.tile([P, H], F32, tag="x1")
            nc.gpsimd.dma_start(x0[:], x_d[0:P, :])
            nc.gpsimd.dma_start(x1[:], x_d[P : 2 * P, :])

            vt0 = const.tile([P, 1], F32, tag="vt0")
            vt1 = const.tile([P, 1], F32, tag="vt1")
            vp0 = const.tile([P, 1], F32, tag="vp0")
            vp1 = const.tile([P, 1], F32, tag="vp1")
            nc.gpsimd.dma_start(vt0[:], vt_d[0:P, :])
            nc.gpsimd.dma_start(vt1[:], vt_d[P : 2 * P, :])
            nc.gpsimd.dma_start(vp0[:], vp_d[0:P, :])
            nc.gpsimd.dma_start(vp1[:], vp_d[P : 2 * P, :])

            ones = const.tile([1, P], F32, tag="ones")
            nc.gpsimd.memset(ones[:], 1.0)

            # s_t / s_p rows: (1, H) = v.T @ x, K=256 split into 2 matmuls
            st_row = const.tile([1, H], F32, tag="st_row")
            sp_row = const.tile([1, H], F32, tag="sp_row")
            for row, v0, v1 in ((st_row, vt0, vt1), (sp_row, vp0, vp1)):
                for jh in range(2):
                    ps = spsum.tile([1, FD], F32, tag="sps")
                    sl = slice(jh * FD, (jh + 1) * FD)
                    nc.tensor.matmul(ps[:], v0[:], x0[:, sl], start=True, stop=False)
                    nc.tensor.matmul(ps[:], v1[:], x1[:, sl], start=False, stop=True)
                    nc.vector.tensor_copy(row[:, sl], ps[:])

            # s_t as per-partition columns: (P, NCHUNK), st_cols[p, c] = s_t[c*P+p]
            st_cols = const.tile([P, NCHUNK], F32, tag="st_cols")
            nc.gpsimd.dma_start(st_scratch[:], st_row[:])
            nc.gpsimd.dma_start(
                st_cols[:],
                st_scratch[:].rearrange("o (c p) -> (o p) c", p=P),
            )

            # s_p broadcast across partitions: (P, H)
            sp_rep = const.tile([P, H], F32, tag="sp_rep")
            for jh in range(2):
                pb = spsum.tile([P, FD], F32, tag="spb")
                sl = slice(jh * FD, (jh + 1) * FD)
                nc.tensor.matmul(pb[:], ones[:], sp_row[:, sl], start=True, stop=True)
                nc.vector.tensor_copy(sp_rep[:, sl], pb[:])

            # ---- main loop: stream edges, PSUM-accumulate over E ----
            for c in range(NCHUNK):
                rows = slice(c * P, (c + 1) * P)
                etiles = []
                for g in range(E // EG):
                    t = epool.tile([P, EG, H], F16, tag="edge")
                    dma_eng = nc.sync if (c * (E // EG) + g) % 2 == 0 else nc.scalar
                    dma_eng.dma_start(t[:], edges_d[c, g])
                    etiles.append(t)

                pss = [
                    mpsum.tile([P, FD], F32, name=f"ps{jh}", tag=f"ps{jh}")
                    for jh in range(2)
                ]
                # e-outer / jh-inner: consecutive matmul pairs share the
                # stationary weights, so the next LDWEIGHTS hides under the
                # paired matmul. Groups for the two PSUM banks interleave.
                for e in range(E):
                    for jh in range(2):
                        sl = slice(jh * FD, (jh + 1) * FD)
                        nc.tensor.matmul(
                            pss[jh][:],
                            wid[:, e, :],
                            etiles[e // EG][:, e % EG, sl],
                            start=(e == 0),
                            stop=(e == E - 1),
                            skip_group_check=True,
                        )

                ot = opool.tile([P, H], F16, tag="ot")
                for jh in range(2):
                    sl = slice(jh * FD, (jh + 1) * FD)
                    nc.vector.scalar_tensor_tensor(
                        out=ot[:, sl],
                        in0=pss[jh][:],
                        scalar=st_cols[:, c : c + 1],
                        in1=sp_rep[:, sl],
                        op0=add,
                        op1=add,
                    )
                nc.gpsimd.dma_start(out_d[rows, :], ot[:])

    nc.compile()
    return nc


def _get_program():
    global _CACHED
    if _CACHED is None:
        _CACHED = _build_program()
    return _CACHED


def kernel(adj, edges, x, Wt, Wp, Wcat, _trace=False):
    del adj  # only its spatial size matters; unused numerically

    edges = np.asarray(edges, dtype=np.float32)
    x = np.asarray(x, dtype=np.float32)
    Wt = np.asarray(Wt, dtype=np.float32)
    Wp = np.asarray(Wp, dtype=np.float32)
    Wcat = np.asarray(Wcat, dtype=np.float32)

    # Fold the 1x1-conv weights: the theta/phi paths collapse to vectors.
    w_e = Wcat[:E]
    v_t = (Wcat[E : E + C] @ Wt).astype(np.float32).reshape(NIN, 1)
    v_p = (Wcat[E + C :] @ Wp).astype(np.float32).reshape(NIN, 1)

    eye = np.eye(P, dtype=np.float32)
    wid = (eye[:, None, :] * w_e[None, :, None]).astype(np.float16)  # (P, E, P)

    # cast to fp16 and relayout to [chunk, group, row, e_local, col] so every
    # device DMA reads fully-contiguous 8 KiB per partition row
    edges16 = edges.astype(np.float16)
    edges16 = edges16.reshape(B, E // EG, EG, NCHUNK, P, H).transpose(0, 3, 1, 4, 2, 5)

    in_maps = []
    for b in range(B):
        in_maps.append(
            {
                "edges": np.ascontiguousarray(edges16[b]),
                "x": np.ascontiguousarray(x[b]),
                "vt": v_t,
                "vp": v_p,
                "wid": wid,
            }
        )

    nc = _get_program()
    res = run_bass_kernel_spmd(nc, in_maps, list(range(N_CORES)), trace=_trace)
    global LAST_RESULT
    LAST_RESULT = res

    out = np.stack([res.results[b]["out"] for b in range(B)])
    return out[:, None, :, :].astype(np.float32)


LAST_RESULT = None
